# revision 3
# baseline (speedup 1.0000x reference)
"""Diagonal low-rank conv (5-tap diagonal stencil + 16x16 channel mix) on 8 TRN2 cores.

out[n,o,h,w] = sum_{i,a} filter_w[o,i,a] * x[n,i,h+a-2,w+a-2]   (zero-padded)

Sharding: data-parallel over batch N=16 -> 2 images per core.

Per-core layout: SBUF partitions = (stripe g in [0,8)) x (channel i in [0,16)),
where stripe g owns image rows [64g, 64(g+1)). A single 128x128 matmul with a
block-diagonal weight (8 copies of W_a^T) then computes one output row for all
8 stripes x 16 channels at once; the 5 diagonal taps accumulate in PSUM. The
rhs of each matmul is one contiguous 512-wide row slice of the padded image,
so input rows stream through SBUF in chunks with no halo re-reads.
"""

import os
import sys
from contextlib import ExitStack

import numpy as np

if "/opt/trn_rl_repo" not in sys.path:
    sys.path.insert(0, "/opt/trn_rl_repo")

import concourse.bass as bass
import concourse.mybir as mybir
import concourse.tile as tile
from concourse import bacc
from concourse.bass_utils import run_bass_kernel_spmd

C = 16          # channels (in == out)
KTAPS = 5       # diagonal taps
PADK = 2        # spatial padding
G = 8           # H-stripes per image (partition groups)
NCORES = 8
CHUNK = 16      # input rows per streamed chunk (per stripe)
RBLK = 8        # output rows staged per store DMA (per stripe)

F32 = mybir.dt.float32
F32R = mybir.dt.float32r


def diag_conv_body(ctx, tc, xin, win, yout, nper, H, W):
    """Emit the per-core kernel. xin: (nper,16,H+4,W+4), win: (5,128,128),
    yout: (nper,16,H,W)."""
    nc = tc.nc
    SH = H // G               # rows per stripe
    Hp, Wp = H + 4, W + 4
    SIN = SH + 4              # input rows needed per stripe (with halo)
    nchunks = (SIN + CHUNK - 1) // CHUNK

    wpool = ctx.enter_context(tc.tile_pool(name="wpool", bufs=1))
    inpool = ctx.enter_context(tc.tile_pool(name="inpool", bufs=4))
    outpool = ctx.enter_context(tc.tile_pool(name="outpool", bufs=3))
    pspool = ctx.enter_context(tc.tile_pool(name="pspool", bufs=8, space="PSUM"))

    # Block-diagonal tap weights, resident for the whole kernel.
    wsb = wpool.tile([128, KTAPS * 128], F32R)
    for a in range(KTAPS):
        nc.sync.dma_start(wsb[:, a * 128:(a + 1) * 128], win[a, :, :])

    def load_chunk(n, c):
        rows = min(CHUNK, SIN - c * CHUNK)
        t = inpool.tile([128, CHUNK * Wp], F32R, tag="inchunk")
        for g in range(G):
            r0 = SH * g + c * CHUNK
            src = xin[n, :, r0:r0 + rows, :].rearrange("i r w -> i (r w)")
            nc.sync.dma_start(t[g * C:(g + 1) * C, :rows * Wp], src)
        return t

    for n in range(nper):
        chunks = {}
        for k in range(SH // CHUNK):          # out-chunks of CHUNK rows
            for cneed in (k, k + 1):
                if cneed not in chunks and cneed < nchunks:
                    chunks[cneed] = load_chunk(n, cneed)
            for half in range(CHUNK // RBLK):  # staging blocks of RBLK rows
                ysb = outpool.tile([128, RBLK * W], F32, tag="ystage")
                for rr in range(RBLK):
                    r = k * CHUNK + half * RBLK + rr
                    ps = pspool.tile([128, W], F32, tag="ps")
                    for a in range(KTAPS):
                        h = r + a
                        ct = chunks[h // CHUNK]
                        off = (h % CHUNK) * Wp + a
                        nc.tensor.matmul(
                            ps[:],
                            wsb[:, a * 128:(a + 1) * 128],
                            ct[:, off:off + W],
                            start=(a == 0),
                            stop=(a == KTAPS - 1),
                        )
                    nc.vector.tensor_copy(ysb[:, rr * W:(rr + 1) * W], ps[:])
                r0 = k * CHUNK + half * RBLK
                for g in range(G):
                    dst = yout[n, :, SH * g + r0:SH * g + r0 + RBLK, :]
                    nc.scalar.dma_start(
                        dst.rearrange("i r w -> i (r w)"), ysb[g * C:(g + 1) * C, :]
                    )


def build_program(nper, H, W):
    nc = bacc.Bacc(trn_type="TRN2")
    xin = nc.dram_tensor("xpad", (nper, C, H + 4, W + 4), F32R, kind="ExternalInput")
    win = nc.dram_tensor("wmat", (KTAPS, 128, 128), F32R, kind="ExternalInput")
    yout = nc.dram_tensor("y", (nper, C, H, W), F32, kind="ExternalOutput")
    with tile.TileContext(nc) as tc:
        with ExitStack() as ctx:
            diag_conv_body(ctx, tc, xin.ap(), win.ap(), yout.ap(), nper, H, W)
    nc.compile()
    return nc


def make_wmat(filter_w):
    """(16,16,5) -> (5,128,128) block-diagonal lhsT (8 copies of W_a^T)."""
    wmat = np.zeros((KTAPS, 128, 128), dtype=np.float32)
    for a in range(KTAPS):
        wt = np.asarray(filter_w[:, :, a], dtype=np.float32).T  # [i, o]
        for g in range(G):
            wmat[a, g * C:(g + 1) * C, g * C:(g + 1) * C] = wt
    return wmat


def run(x, filter_w, trace=False, tmpdir=None):
    """Returns (full output, BassKernelResults)."""
    x = np.asarray(x, dtype=np.float32)
    filter_w = np.asarray(filter_w, dtype=np.float32)
    N, _, H, W = x.shape
    nper = N // NCORES

    xpad = np.zeros((N, C, H + 4, W + 4), dtype=np.float32)
    xpad[:, :, PADK:PADK + H, PADK:PADK + W] = x
    wmat = make_wmat(filter_w)

    nc = build_program(nper, H, W)
    in_maps = [
        {"xpad": np.ascontiguousarray(xpad[c * nper:(c + 1) * nper]), "wmat": wmat}
        for c in range(NCORES)
    ]
    res = run_bass_kernel_spmd(
        nc, in_maps, list(range(NCORES)), trace=trace, tmpdir=tmpdir
    )
    out = np.concatenate([res.results[c]["y"] for c in range(NCORES)], axis=0)
    return out, res


def kernel(x, filter_w):
    return run(x, filter_w)[0]
